# revision 20
# baseline (speedup 1.0000x reference)
"""BatchGraphTripleConv on 8 TRN2 NeuronCores.

Data-parallel over batch B=64 (8 graphs per core). Gather/scatter are
expressed as one-hot matmuls on the TensorEngine:
  - gather:  cur_s.T-contribution = A_s.T @ Gt_s  where Gt_s[n,t] = [s_idx[t]==n]
  - scatter: pooled.T = new_s.T @ Gs + new_o.T @ Go where Gs[t,n] = [s_idx[t]==n]/cnt[n]
The mean-pool division is folded into the scatter one-hot on the host.
All matmuls run in bf16 with f32 PSUM accumulation; layouts are chosen so
no on-chip transpose is ever needed.
"""

import numpy as np
import ml_dtypes

import concourse.bacc as bacc
import concourse.mybir as mybir
import concourse.tile as tile
from concourse.bass import ts
from concourse.bass_utils import run_bass_kernel_spmd

B, N, T = 64, 256, 512
DO, DP, H, DOUT = 512, 512, 512, 512
N_CORES = 8
BL = B // N_CORES  # batch elems per core
P = 128
BF16 = mybir.dt.bfloat16
F32 = mybir.dt.float32
NPBF16 = ml_dtypes.bfloat16

KD = DO // P   # 4 k-chunks over feature dims (512)
MN = N // P    # 2 m-chunks over nodes (256)
MT = T // P    # 4 m-chunks over triples (512)

# dev knobs (unused by the grading path)
TRACE = False
LAST_EXEC_TIME_NS = None
_PROGRAM_CACHE = {}


def _build_program(with_b2: bool, with_nb2: bool):
    nc = bacc.Bacc(
        "TRN2",
        target_bir_lowering=False,
        debug=False,
        num_devices=N_CORES,
    )

    def din(name, shape, dt=BF16):
        return nc.dram_tensor(name, shape, dt, kind="ExternalInput").ap()

    # per-core batch shard, host pre-tiled to [BL, 128, chunks, free]
    objT_d = din("objT", [BL, P, KD, N])          # obj[b].T tiled: [p,k,n] = obj[b, n, k*128+p]
    predT_d = din("predT", [BL, P, KD, T])        # pred[b].T tiled
    gts_d = din("gts", [BL, P, MN, T])            # gather one-hot (subject): [p,kn,t] = [s_idx==kn*128+p]
    gto_d = din("gto", [BL, P, MN, T])
    gss_d = din("gss", [BL, P, MT, N])            # scatter one-hot / count: [p,tk,n]
    gos_d = din("gos", [BL, P, MT, N])
    # weights (same on every core)
    w1s_d = din("w1s", [P, KD, H])
    w1p_d = din("w1p", [P, KD, H])
    w1o_d = din("w1o", [P, KD, H])
    w2_d = din("w2", [P, KD, 2 * H + DOUT])
    n2w1_d = din("n2w1", [P, KD, H])
    n2w2_d = din("n2w2", [P, KD, DOUT])
    b1_d = din("b1", [P, KD], F32)
    n2b1_d = din("n2b1", [P, KD], F32)
    if with_b2:
        e0_d = din("e0", [P, P])                  # row0 = ones
        b2pad_d = din("b2pad", [P, 3, H])         # row0 = n1_b2 thirds
    if with_nb2:
        e0b_d = din("e0b", [P, P])
        nb2pad_d = din("nb2pad", [P, DOUT])       # row0 = n2_b2

    out_obj_d = nc.dram_tensor("out_obj", [BL, N, DOUT], F32, kind="ExternalOutput").ap()
    out_p_d = nc.dram_tensor("out_p", [BL, T, DOUT], F32, kind="ExternalOutput").ap()

    RELU = mybir.ActivationFunctionType.Relu

    with tile.TileContext(nc) as tc:
        with (
            tc.tile_pool(name="wpool", bufs=1) as wp,
            tc.tile_pool(name="inpool", bufs=2) as ip,
            tc.tile_pool(name="midpool", bufs=2) as mp,
            tc.tile_pool(name="outpool", bufs=2) as op,
            tc.tile_pool(name="pspool", bufs=8, space="PSUM") as pp,
        ):
            # ---- resident weights ----
            # Criticality-ordered DMA: the first stage-A matmuls need only
            # w1s/w1o k-chunk 0 and objT[0] k-chunk 0, so those stream first
            # (per-k granularity); everything not needed until stage B+ is
            # emitted after stage A so it overlaps compute.
            w1s = wp.tile([P, KD, H], BF16)
            w1p = wp.tile([P, KD, H], BF16)
            w1o = wp.tile([P, KD, H], BF16)
            w2 = wp.tile([P, KD, 2 * H + DOUT], BF16)
            n2w1 = wp.tile([P, KD, H], BF16)
            n2w2 = wp.tile([P, KD, DOUT], BF16)
            b1 = wp.tile([P, KD], F32)
            n2b1 = wp.tile([P, KD], F32)

            def alloc_inputs(b):
                return {
                    "objT": ip.tile([P, KD, N], BF16, tag="objT", name=f"objT{b}"),
                    "predT": ip.tile([P, KD, T], BF16, tag="predT", name=f"predT{b}"),
                    "gts": ip.tile([P, MN, T], BF16, tag="gts", name=f"gts{b}"),
                    "gto": ip.tile([P, MN, T], BF16, tag="gto", name=f"gto{b}"),
                    "gss": ip.tile([P, MT, N], BF16, tag="gss", name=f"gss{b}"),
                    "gos": ip.tile([P, MT, N], BF16, tag="gos", name=f"gos{b}"),
                }

            def load_inputs(b, tl):
                nc.sync.dma_start(out=tl["objT"][:], in_=objT_d[b])
                nc.sync.dma_start(out=tl["gts"][:], in_=gts_d[b])
                nc.sync.dma_start(out=tl["gto"][:], in_=gto_d[b])
                nc.sync.dma_start(out=tl["predT"][:], in_=predT_d[b])
                nc.sync.dma_start(out=tl["gss"][:], in_=gss_d[b])
                nc.sync.dma_start(out=tl["gos"][:], in_=gos_d[b])

            # HAM warmup: dependency-free dummy matmuls keep the PE busy during
            # the DMA-paced head so the clock gate reaches 2.4 GHz before the
            # dense phase. Each costs ~27-53ns and only runs when PE is idle.
            warm = wp.tile([P, P], BF16)
            nc.gpsimd.memset(warm[:], 0)
            ps_w = pp.tile([P, 512], F32, tag="ps", name="warmps")

            def warm_mms(n):
                for _ in range(n):
                    nc.tensor.matmul(out=ps_w[:, :64], lhsT=warm[:], rhs=warm[:, :64],
                                     start=True, stop=True, skip_group_check=True)

            def stage_a(objT, warming=False):
                As = mp.tile([P, MN, H], BF16, tag="As")
                Ao = mp.tile([P, MN, H], BF16, tag="Ao")
                for m in range(MN):
                    ps_s = pp.tile([P, 512], F32, tag="ps")
                    ps_o = pp.tile([P, 512], F32, tag="ps")
                    for k in range(KD):
                        lhsT = objT[:, k, ts(m, P)]
                        nc.tensor.matmul(out=ps_s[:], lhsT=lhsT, rhs=w1s[:, k, :],
                                         start=(k == 0), stop=(k == KD - 1))
                        nc.tensor.matmul(out=ps_o[:], lhsT=lhsT, rhs=w1o[:, k, :],
                                         start=(k == 0), stop=(k == KD - 1))
                        if warming:
                            warm_mms(3)
                    nc.vector.tensor_copy(out=As[:, m, :], in_=ps_s[:])
                    nc.vector.tensor_copy(out=Ao[:, m, :], in_=ps_o[:])
                return As, Ao

            # critical path: per-k interleaved loads of stage-A operands for b=0
            inputs_tl = {0: alloc_inputs(0), 1: alloc_inputs(1)}
            in0, in1 = inputs_tl[0], inputs_tl[1]
            for k in range(KD):
                nc.sync.dma_start(out=w1s[:, k, :], in_=w1s_d[:, k, :])
                nc.sync.dma_start(out=w1o[:, k, :], in_=w1o_d[:, k, :])
                nc.sync.dma_start(out=in0["objT"][:, k, :], in_=objT_d[0, :, k, :])
            nc.sync.dma_start(out=in1["objT"][:], in_=objT_d[1])
            nc.sync.dma_start(out=in0["gts"][:], in_=gts_d[0])
            nc.sync.dma_start(out=in0["gto"][:], in_=gto_d[0])
            nc.sync.dma_start(out=in0["predT"][:], in_=predT_d[0])
            nc.sync.dma_start(out=w1p[:], in_=w1p_d[:])

            # stage A for b=0 and b=1 back-to-back: PE work that covers the
            # stream-in of stage B/C inputs.
            warm_mms(100)
            first_stage = {0: stage_a(in0["objT"], warming=True), 1: stage_a(in1["objT"])}

            # deferred bulk weights: needed from stage B/C/E/F onward
            nc.sync.dma_start(out=b1[:], in_=b1_d[:])
            nc.sync.dma_start(out=w2[:], in_=w2_d[:])
            nc.sync.dma_start(out=in0["gss"][:], in_=gss_d[0])
            nc.sync.dma_start(out=in0["gos"][:], in_=gos_d[0])
            nc.sync.dma_start(out=n2w1[:], in_=n2w1_d[:])
            nc.sync.dma_start(out=n2w2[:], in_=n2w2_d[:])
            nc.sync.dma_start(out=n2b1[:], in_=n2b1_d[:])
            nc.sync.dma_start(out=in1["gts"][:], in_=gts_d[1])
            nc.sync.dma_start(out=in1["gto"][:], in_=gto_d[1])
            nc.sync.dma_start(out=in1["predT"][:], in_=predT_d[1])
            nc.sync.dma_start(out=in1["gss"][:], in_=gss_d[1])
            nc.sync.dma_start(out=in1["gos"][:], in_=gos_d[1])
            if with_b2:
                e0 = wp.tile([P, P], BF16)
                b2pad = wp.tile([P, 3, H], BF16)
                nc.sync.dma_start(out=e0[:], in_=e0_d[:])
                nc.sync.dma_start(out=b2pad[:], in_=b2pad_d[:])
            if with_nb2:
                e0b = wp.tile([P, P], BF16)
                nb2pad = wp.tile([P, DOUT], BF16)
                nc.sync.dma_start(out=e0b[:], in_=e0b_d[:])
                nc.sync.dma_start(out=nb2pad[:], in_=nb2pad_d[:])

            # ---- per-batch stage emitters (pairwise-interleaved below) ----
            def stage_b(tl, As, Ao):
                # hid.T = relu(A_s.T@Gt_s + A_o.T@Gt_o + W1_p.T@pred.T + b1);
                # gather MMs (need only gts/gto) before pred MMs (predT/w1p)
                hidT = mp.tile([P, KD, T], BF16, tag="hidT")
                ps_hid = []
                for hm in range(KD):
                    ps = pp.tile([P, 512], F32, tag="ps", name=f"ps_hid{hm}")
                    ps_hid.append(ps)
                    for kn in range(MN):
                        nc.tensor.matmul(out=ps[:], lhsT=As[:, kn, ts(hm, P)],
                                         rhs=tl["gts"][:, kn, :], start=(kn == 0), stop=False)
                        nc.tensor.matmul(out=ps[:], lhsT=Ao[:, kn, ts(hm, P)],
                                         rhs=tl["gto"][:, kn, :], start=False, stop=False)
                for hm in range(KD):
                    ps = ps_hid[hm]
                    for kp in range(KD):
                        nc.tensor.matmul(out=ps[:], lhsT=w1p[:, kp, ts(hm, P)],
                                         rhs=tl["predT"][:, kp, :],
                                         start=False, stop=(kp == KD - 1))
                    nc.scalar.activation(out=hidT[:, hm, :], in_=ps[:], func=RELU,
                                         bias=b1[:, hm : hm + 1])
                return hidT

            def stage_c(b, hidT):
                # new_t thirds, T-major: new_x = relu(hid @ W2_x + b2_x)
                ns = mp.tile([P, MT, H], BF16, tag="ns")
                no = mp.tile([P, MT, H], BF16, tag="no")
                npf = op.tile([P, MT, DOUT], F32, tag="npf")
                for tm in range(MT):
                    for third, dst in ((0, ns), (1, None), (2, no)):
                        ps = pp.tile([P, 512], F32, tag="ps")
                        for hk in range(KD):
                            nc.tensor.matmul(out=ps[:], lhsT=hidT[:, hk, ts(tm, P)],
                                             rhs=w2[:, hk, ts(third, 512)],
                                             start=(hk == 0),
                                             stop=(hk == KD - 1 and not with_b2))
                        if with_b2:
                            nc.tensor.matmul(out=ps[:], lhsT=e0[:], rhs=b2pad[:, third, :],
                                             start=False, stop=True)
                        # split evacuations across ScalarE and VectorE so the
                        # PSUM banks recycle fast enough to never stall the PE
                        if dst is None:
                            nc.vector.tensor_scalar_max(out=npf[:, tm, :], in0=ps[:], scalar1=0.0)
                        elif dst is no:
                            nc.vector.tensor_scalar_max(out=dst[:, tm, :], in0=ps[:], scalar1=0.0)
                        else:
                            nc.scalar.activation(out=dst[:, tm, :], in_=ps[:], func=RELU)
                nc.sync.dma_start(
                    out=out_p_d[b].rearrange("(tm p) j -> p tm j", p=P), in_=npf[:]
                )
                return ns, no

            def stage_d(tl, ns, no):
                # pooled.T = new_s.T @ Gs + new_o.T @ Go  (count-scaled)
                pooledT = mp.tile([P, KD, N], BF16, tag="pooledT")
                for hm in range(KD):
                    ps = pp.tile([P, 256], F32, tag="ps")
                    for tk in range(MT):
                        nc.tensor.matmul(out=ps[:], lhsT=ns[:, tk, ts(hm, P)],
                                         rhs=tl["gss"][:, tk, :], start=(tk == 0), stop=False)
                        nc.tensor.matmul(out=ps[:], lhsT=no[:, tk, ts(hm, P)],
                                         rhs=tl["gos"][:, tk, :],
                                         start=False, stop=(tk == MT - 1))
                    nc.vector.tensor_copy(out=pooledT[:, hm, :], in_=ps[:])
                return pooledT

            def stage_e(pooledT):
                # h2.T = relu(n2w1.T @ pooled.T + n2b1)
                h2T = mp.tile([P, KD, N], BF16, tag="h2T")
                for m2 in range(KD):
                    ps = pp.tile([P, 256], F32, tag="ps")
                    for hk in range(KD):
                        nc.tensor.matmul(out=ps[:], lhsT=n2w1[:, hk, ts(m2, P)],
                                         rhs=pooledT[:, hk, :],
                                         start=(hk == 0), stop=(hk == KD - 1))
                    nc.scalar.activation(out=h2T[:, m2, :], in_=ps[:], func=RELU,
                                         bias=n2b1[:, m2 : m2 + 1])
                return h2T

            def stage_f(b, h2T):
                # out_obj = relu(h2 @ n2w2 + n2b2)  (node-major)
                oo = op.tile([P, MN, DOUT], F32, tag="oo")
                oo_dram = out_obj_d[b].rearrange("(nm p) j -> p nm j", p=P)
                for nm in range(MN):
                    ps = pp.tile([P, 512], F32, tag="ps")
                    for k2 in range(KD):
                        nc.tensor.matmul(out=ps[:], lhsT=h2T[:, k2, ts(nm, P)],
                                         rhs=n2w2[:, k2, :],
                                         start=(k2 == 0),
                                         stop=(k2 == KD - 1 and not with_nb2))
                    if with_nb2:
                        nc.tensor.matmul(out=ps[:], lhsT=e0b[:], rhs=nb2pad[:],
                                         start=False, stop=True)
                    nc.scalar.activation(out=oo[:, nm, :], in_=ps[:], func=RELU)
                    nc.sync.dma_start(out=oo_dram[:, nm, :], in_=oo[:, nm, :])

            for b in range(BL):
                if b not in inputs_tl:
                    inputs_tl[b] = alloc_inputs(b)
                    load_inputs(b, inputs_tl[b])
                tl = inputs_tl.pop(b)
                if b in first_stage:
                    As, Ao = first_stage.pop(b)
                else:
                    As, Ao = stage_a(tl["objT"])
                hidT = stage_b(tl, As, Ao)
                ns, no = stage_c(b, hidT)
                pooledT = stage_d(tl, ns, no)
                h2T = stage_e(pooledT)
                stage_f(b, h2T)

    nc.compile()
    return nc


def _tile_kx(a):
    """[R, F] -> [128, R//128, F] with [p, k, f] = a[k*128+p, f], contiguous."""
    r, f = a.shape
    return np.ascontiguousarray(a.reshape(r // P, P, f).transpose(1, 0, 2))


def kernel(**inputs):
    obj = np.asarray(inputs["obj_vecs"], dtype=np.float32)    # [B, N, DO]
    pred = np.asarray(inputs["pred_vecs"], dtype=np.float32)  # [B, T, DP]
    edges = np.asarray(inputs["edges"])                       # [B, T, 3] int
    n1_w1 = np.asarray(inputs["n1_w1"], dtype=np.float32)     # [2DO+DP, H]
    n1_b1 = np.asarray(inputs["n1_b1"], dtype=np.float32)
    n1_w2 = np.asarray(inputs["n1_w2"], dtype=np.float32)     # [H, 2H+DOUT]
    n1_b2 = np.asarray(inputs["n1_b2"], dtype=np.float32)
    n2_w1 = np.asarray(inputs["n2_w1"], dtype=np.float32)
    n2_b1 = np.asarray(inputs["n2_b1"], dtype=np.float32)
    n2_w2 = np.asarray(inputs["n2_w2"], dtype=np.float32)
    n2_b2 = np.asarray(inputs["n2_b2"], dtype=np.float32)

    s_idx = edges[:, :, 0].astype(np.int64)  # [B, T]
    o_idx = edges[:, :, 2].astype(np.int64)

    # one-hots + per-node incidence counts (mean-pool scaling folded into scatter)
    ar_n = np.arange(N)
    gt_s = (s_idx[:, None, :] == ar_n[None, :, None])  # [B, N, T] bool
    gt_o = (o_idx[:, None, :] == ar_n[None, :, None])
    cnt = gt_s.sum(axis=2) + gt_o.sum(axis=2)          # [B, N]
    rcnt = 1.0 / np.maximum(cnt, 1).astype(np.float32)
    g_s = gt_s.transpose(0, 2, 1) * rcnt[:, None, :]   # [B, T, N] f32, scaled
    g_o = gt_o.transpose(0, 2, 1) * rcnt[:, None, :]

    # host tiling into [B, 128, chunks, free] bf16
    def tile_batch(a):  # [B, R, F] -> [B, 128, R//128, F]
        bsz, r, f = a.shape
        return np.ascontiguousarray(
            a.reshape(bsz, r // P, P, f).transpose(0, 2, 1, 3).astype(NPBF16)
        )

    objT_t = tile_batch(obj.transpose(0, 2, 1))   # [B,128,KD,N]
    predT_t = tile_batch(pred.transpose(0, 2, 1)) # [B,128,KD,T]
    gts_t = tile_batch(gt_s.astype(np.float32))   # [B,128,MN,T]
    gto_t = tile_batch(gt_o.astype(np.float32))
    gss_t = tile_batch(g_s)                       # [B,128,MT,N]
    gos_t = tile_batch(g_o)

    w1s_t = _tile_kx(n1_w1[0:DO]).astype(NPBF16)
    w1p_t = _tile_kx(n1_w1[DO : DO + DP]).astype(NPBF16)
    w1o_t = _tile_kx(n1_w1[DO + DP :]).astype(NPBF16)
    w2_t = _tile_kx(n1_w2).astype(NPBF16)
    n2w1_t = _tile_kx(n2_w1).astype(NPBF16)
    n2w2_t = _tile_kx(n2_w2).astype(NPBF16)
    b1_t = np.ascontiguousarray(n1_b1.reshape(KD, P).T)    # [128, KD] f32
    n2b1_t = np.ascontiguousarray(n2_b1.reshape(KD, P).T)

    with_b2 = bool(np.any(n1_b2 != 0))
    with_nb2 = bool(np.any(n2_b2 != 0))

    key = (with_b2, with_nb2)
    if key not in _PROGRAM_CACHE:
        _PROGRAM_CACHE[key] = _build_program(with_b2, with_nb2)
    nc = _PROGRAM_CACHE[key]

    in_maps = []
    for c in range(N_CORES):
        sl = slice(c * BL, (c + 1) * BL)
        m = {
            "objT": objT_t[sl], "predT": predT_t[sl],
            "gts": gts_t[sl], "gto": gto_t[sl],
            "gss": gss_t[sl], "gos": gos_t[sl],
            "w1s": w1s_t, "w1p": w1p_t, "w1o": w1o_t, "w2": w2_t,
            "n2w1": n2w1_t, "n2w2": n2w2_t, "b1": b1_t, "n2b1": n2b1_t,
        }
        if with_b2:
            e0 = np.zeros((P, P), NPBF16); e0[0, :] = 1
            m["e0"] = e0
            b2pad = np.zeros((P, 3, H), np.float32)
            b2pad[0, 0, :] = n1_b2[0:H]
            b2pad[0, 1, :] = n1_b2[H : H + DOUT]
            b2pad[0, 2, :] = n1_b2[H + DOUT :]
            m["b2pad"] = b2pad.astype(NPBF16)
        if with_nb2:
            e0b = np.zeros((P, P), NPBF16); e0b[0, :] = 1
            m["e0b"] = e0b
            nb2 = np.zeros((P, DOUT), np.float32); nb2[0, :] = n2_b2
            m["nb2pad"] = nb2.astype(NPBF16)
        in_maps.append(m)

    res = run_bass_kernel_spmd(nc, in_maps, core_ids=list(range(N_CORES)), trace=TRACE)
    global LAST_EXEC_TIME_NS
    LAST_EXEC_TIME_NS = res.exec_time_ns

    new_obj = np.empty((B, N, DOUT), np.float32)
    new_p = np.empty((B, T, DOUT), np.float32)
    for c in range(N_CORES):
        sl = slice(c * BL, (c + 1) * BL)
        new_obj[sl] = res.results[c]["out_obj"]
        new_p[sl] = res.results[c]["out_p"]
    return new_obj, new_p


# revision 23
# speedup vs baseline: 1.0039x; 1.0039x over previous
"""BatchGraphTripleConv on 8 TRN2 NeuronCores.

Data-parallel over batch B=64 (8 graphs per core). Gather/scatter are
expressed as one-hot matmuls on the TensorEngine:
  - gather:  cur_s.T-contribution = A_s.T @ Gt_s  where Gt_s[n,t] = [s_idx[t]==n]
  - scatter: pooled.T = new_s.T @ Gs + new_o.T @ Go where Gs[t,n] = [s_idx[t]==n]/cnt[n]
The mean-pool division is folded into the scatter one-hot on the host.
All matmuls run in bf16 with f32 PSUM accumulation; layouts are chosen so
no on-chip transpose is ever needed.
"""

import numpy as np
import ml_dtypes

import concourse.bacc as bacc
import concourse.mybir as mybir
import concourse.tile as tile
from concourse.bass import ts
from concourse.bass_utils import run_bass_kernel_spmd

B, N, T = 64, 256, 512
DO, DP, H, DOUT = 512, 512, 512, 512
N_CORES = 8
BL = B // N_CORES  # batch elems per core
P = 128
BF16 = mybir.dt.bfloat16
F32 = mybir.dt.float32
NPBF16 = ml_dtypes.bfloat16

KD = DO // P   # 4 k-chunks over feature dims (512)
MN = N // P    # 2 m-chunks over nodes (256)
MT = T // P    # 4 m-chunks over triples (512)

# dev knobs (unused by the grading path)
TRACE = False
LAST_EXEC_TIME_NS = None
_PROGRAM_CACHE = {}


def _build_program(with_b2: bool, with_nb2: bool):
    nc = bacc.Bacc(
        "TRN2",
        target_bir_lowering=False,
        debug=False,
        num_devices=N_CORES,
    )

    def din(name, shape, dt=BF16):
        return nc.dram_tensor(name, shape, dt, kind="ExternalInput").ap()

    # per-core batch shard, host pre-tiled to [BL, 128, chunks, free]
    objT_d = din("objT", [BL, P, KD, N])          # obj[b].T tiled: [p,k,n] = obj[b, n, k*128+p]
    predT_d = din("predT", [BL, P, KD, T])        # pred[b].T tiled
    gts_d = din("gts", [BL, P, MN, T])            # gather one-hot (subject): [p,kn,t] = [s_idx==kn*128+p]
    gto_d = din("gto", [BL, P, MN, T])
    gss_d = din("gss", [BL, P, MT, N])            # scatter one-hot / count: [p,tk,n]
    gos_d = din("gos", [BL, P, MT, N])
    # weights (same on every core)
    w1s_d = din("w1s", [P, KD, H])
    w1p_d = din("w1p", [P, KD, H])
    w1o_d = din("w1o", [P, KD, H])
    w2_d = din("w2", [P, KD, 2 * H + DOUT])
    n2w1_d = din("n2w1", [P, KD, H])
    n2w2_d = din("n2w2", [P, KD, DOUT])
    b1_d = din("b1", [P, KD], F32)
    n2b1_d = din("n2b1", [P, KD], F32)
    if with_b2:
        e0_d = din("e0", [P, P])                  # row0 = ones
        b2pad_d = din("b2pad", [P, 3, H])         # row0 = n1_b2 thirds
    if with_nb2:
        e0b_d = din("e0b", [P, P])
        nb2pad_d = din("nb2pad", [P, DOUT])       # row0 = n2_b2

    out_obj_d = nc.dram_tensor("out_obj", [BL, N, DOUT], F32, kind="ExternalOutput").ap()
    out_p_d = nc.dram_tensor("out_p", [BL, T, DOUT], F32, kind="ExternalOutput").ap()

    RELU = mybir.ActivationFunctionType.Relu

    with tile.TileContext(nc) as tc:
        with (
            tc.tile_pool(name="wpool", bufs=1) as wp,
            tc.tile_pool(name="inpool", bufs=2) as ip,
            tc.tile_pool(name="midpool", bufs=2) as mp,
            tc.tile_pool(name="outpool", bufs=2) as op,
            tc.tile_pool(name="pspool", bufs=8, space="PSUM") as pp,
        ):
            # ---- resident weights ----
            # Criticality-ordered DMA: the first stage-A matmuls need only
            # w1s/w1o k-chunk 0 and objT[0] k-chunk 0, so those stream first
            # (per-k granularity); everything not needed until stage B+ is
            # emitted after stage A so it overlaps compute.
            w1s = wp.tile([P, KD, H], BF16)
            w1p = wp.tile([P, KD, H], BF16)
            w1o = wp.tile([P, KD, H], BF16)
            w2 = wp.tile([P, KD, 2 * H + DOUT], BF16)
            n2w1 = wp.tile([P, KD, H], BF16)
            n2w2 = wp.tile([P, KD, DOUT], BF16)
            b1 = wp.tile([P, KD], F32)
            n2b1 = wp.tile([P, KD], F32)

            def alloc_inputs(b):
                return {
                    "objT": ip.tile([P, KD, N], BF16, tag="objT", name=f"objT{b}"),
                    "predT": ip.tile([P, KD, T], BF16, tag="predT", name=f"predT{b}"),
                    "gts": ip.tile([P, MN, T], BF16, tag="gts", name=f"gts{b}"),
                    "gto": ip.tile([P, MN, T], BF16, tag="gto", name=f"gto{b}"),
                    "gss": ip.tile([P, MT, N], BF16, tag="gss", name=f"gss{b}"),
                    "gos": ip.tile([P, MT, N], BF16, tag="gos", name=f"gos{b}"),
                }

            def load_inputs(b, tl):
                nc.sync.dma_start(out=tl["objT"][:], in_=objT_d[b])
                nc.sync.dma_start(out=tl["gts"][:], in_=gts_d[b])
                nc.sync.dma_start(out=tl["gto"][:], in_=gto_d[b])
                nc.sync.dma_start(out=tl["predT"][:], in_=predT_d[b])
                nc.sync.dma_start(out=tl["gss"][:], in_=gss_d[b])
                nc.sync.dma_start(out=tl["gos"][:], in_=gos_d[b])

            # HAM warmup: dependency-free dummy matmuls keep the PE busy during
            # the DMA-paced head so the clock gate reaches 2.4 GHz before the
            # dense phase. Each costs ~27-53ns and only runs when PE is idle.
            warm = wp.tile([P, P], BF16)
            nc.gpsimd.memset(warm[:], 0)
            ps_w = pp.tile([P, 512], F32, tag="ps", name="warmps")

            def warm_mms(n):
                for _ in range(n):
                    nc.tensor.matmul(out=ps_w[:, :64], lhsT=warm[:], rhs=warm[:, :64],
                                     start=True, stop=True, skip_group_check=True)

            def stage_a(objT, warming=False):
                # s-pass before o-pass: the first matmuls need only w1s chunks
                # + objT chunks, shrinking the b=0 critical DMA set; w1o
                # streams in behind the s-pass.
                As = mp.tile([P, MN, H], BF16, tag="As")
                Ao = mp.tile([P, MN, H], BF16, tag="Ao")
                for m in range(MN):
                    ps_s = pp.tile([P, 512], F32, tag="ps")
                    for k in range(KD):
                        nc.tensor.matmul(out=ps_s[:], lhsT=objT[:, k, ts(m, P)],
                                         rhs=w1s[:, k, :],
                                         start=(k == 0), stop=(k == KD - 1))
                        if warming:
                            warm_mms(3)
                    nc.vector.tensor_copy(out=As[:, m, :], in_=ps_s[:])
                for m in range(MN):
                    ps_o = pp.tile([P, 512], F32, tag="ps")
                    for k in range(KD):
                        nc.tensor.matmul(out=ps_o[:], lhsT=objT[:, k, ts(m, P)],
                                         rhs=w1o[:, k, :],
                                         start=(k == 0), stop=(k == KD - 1))
                    nc.vector.tensor_copy(out=Ao[:, m, :], in_=ps_o[:])
                return As, Ao

            # critical path: per-k interleaved loads of stage-A operands for b=0
            inputs_tl = {0: alloc_inputs(0), 1: alloc_inputs(1)}
            in0, in1 = inputs_tl[0], inputs_tl[1]
            for k in range(KD):
                nc.sync.dma_start(out=w1s[:, k, :], in_=w1s_d[:, k, :])
                nc.sync.dma_start(out=in0["objT"][:, k, :], in_=objT_d[0, :, k, :])
            for k in range(KD):
                nc.sync.dma_start(out=w1o[:, k, :], in_=w1o_d[:, k, :])
            nc.sync.dma_start(out=in1["objT"][:], in_=objT_d[1])
            nc.sync.dma_start(out=in0["gts"][:], in_=gts_d[0])
            nc.sync.dma_start(out=in0["gto"][:], in_=gto_d[0])
            nc.sync.dma_start(out=in0["predT"][:], in_=predT_d[0])
            nc.sync.dma_start(out=w1p[:], in_=w1p_d[:])

            # stage A for b=0 and b=1 back-to-back: PE work that covers the
            # stream-in of stage B/C inputs.
            warm_mms(100)
            first_stage = {0: stage_a(in0["objT"], warming=True), 1: stage_a(in1["objT"])}

            # deferred bulk weights: needed from stage B/C/E/F onward
            nc.sync.dma_start(out=b1[:], in_=b1_d[:])
            nc.sync.dma_start(out=w2[:], in_=w2_d[:])
            nc.sync.dma_start(out=in0["gss"][:], in_=gss_d[0])
            nc.sync.dma_start(out=in0["gos"][:], in_=gos_d[0])
            nc.sync.dma_start(out=n2w1[:], in_=n2w1_d[:])
            nc.sync.dma_start(out=n2w2[:], in_=n2w2_d[:])
            nc.sync.dma_start(out=n2b1[:], in_=n2b1_d[:])
            nc.sync.dma_start(out=in1["gts"][:], in_=gts_d[1])
            nc.sync.dma_start(out=in1["gto"][:], in_=gto_d[1])
            nc.sync.dma_start(out=in1["predT"][:], in_=predT_d[1])
            nc.sync.dma_start(out=in1["gss"][:], in_=gss_d[1])
            nc.sync.dma_start(out=in1["gos"][:], in_=gos_d[1])
            if with_b2:
                e0 = wp.tile([P, P], BF16)
                b2pad = wp.tile([P, 3, H], BF16)
                nc.sync.dma_start(out=e0[:], in_=e0_d[:])
                nc.sync.dma_start(out=b2pad[:], in_=b2pad_d[:])
            if with_nb2:
                e0b = wp.tile([P, P], BF16)
                nb2pad = wp.tile([P, DOUT], BF16)
                nc.sync.dma_start(out=e0b[:], in_=e0b_d[:])
                nc.sync.dma_start(out=nb2pad[:], in_=nb2pad_d[:])

            # ---- per-batch stage emitters (pairwise-interleaved below) ----
            def stage_b(tl, As, Ao):
                # hid.T = relu(A_s.T@Gt_s + A_o.T@Gt_o + W1_p.T@pred.T + b1);
                # gather MMs (need only gts/gto) before pred MMs (predT/w1p)
                hidT = mp.tile([P, KD, T], BF16, tag="hidT")
                ps_hid = []
                for hm in range(KD):
                    ps = pp.tile([P, 512], F32, tag="ps", name=f"ps_hid{hm}")
                    ps_hid.append(ps)
                    for kn in range(MN):
                        nc.tensor.matmul(out=ps[:], lhsT=As[:, kn, ts(hm, P)],
                                         rhs=tl["gts"][:, kn, :], start=(kn == 0), stop=False)
                        nc.tensor.matmul(out=ps[:], lhsT=Ao[:, kn, ts(hm, P)],
                                         rhs=tl["gto"][:, kn, :], start=False, stop=False)
                for hm in range(KD):
                    ps = ps_hid[hm]
                    for kp in range(KD):
                        nc.tensor.matmul(out=ps[:], lhsT=w1p[:, kp, ts(hm, P)],
                                         rhs=tl["predT"][:, kp, :],
                                         start=False, stop=(kp == KD - 1))
                    nc.scalar.activation(out=hidT[:, hm, :], in_=ps[:], func=RELU,
                                         bias=b1[:, hm : hm + 1])
                return hidT

            def stage_c(b, hidT):
                # new_t thirds, T-major: new_x = relu(hid @ W2_x + b2_x)
                ns = mp.tile([P, MT, H], BF16, tag="ns")
                no = mp.tile([P, MT, H], BF16, tag="no")
                npf = op.tile([P, MT, DOUT], F32, tag="npf")
                for tm in range(MT):
                    for third, dst in ((0, ns), (1, None), (2, no)):
                        ps = pp.tile([P, 512], F32, tag="ps")
                        for hk in range(KD):
                            nc.tensor.matmul(out=ps[:], lhsT=hidT[:, hk, ts(tm, P)],
                                             rhs=w2[:, hk, ts(third, 512)],
                                             start=(hk == 0),
                                             stop=(hk == KD - 1 and not with_b2))
                        if with_b2:
                            nc.tensor.matmul(out=ps[:], lhsT=e0[:], rhs=b2pad[:, third, :],
                                             start=False, stop=True)
                        if dst is None:
                            nc.scalar.activation(out=npf[:, tm, :], in_=ps[:], func=RELU)
                        else:
                            nc.scalar.activation(out=dst[:, tm, :], in_=ps[:], func=RELU)
                nc.sync.dma_start(
                    out=out_p_d[b].rearrange("(tm p) j -> p tm j", p=P), in_=npf[:]
                )
                return ns, no

            def stage_d(tl, ns, no):
                # pooled.T = new_s.T @ Gs + new_o.T @ Go  (count-scaled)
                pooledT = mp.tile([P, KD, N], BF16, tag="pooledT")
                for hm in range(KD):
                    ps = pp.tile([P, 256], F32, tag="ps")
                    for tk in range(MT):
                        nc.tensor.matmul(out=ps[:], lhsT=ns[:, tk, ts(hm, P)],
                                         rhs=tl["gss"][:, tk, :], start=(tk == 0), stop=False)
                        nc.tensor.matmul(out=ps[:], lhsT=no[:, tk, ts(hm, P)],
                                         rhs=tl["gos"][:, tk, :],
                                         start=False, stop=(tk == MT - 1))
                    nc.vector.tensor_copy(out=pooledT[:, hm, :], in_=ps[:])
                return pooledT

            def stage_e(pooledT):
                # h2.T = relu(n2w1.T @ pooled.T + n2b1)
                h2T = mp.tile([P, KD, N], BF16, tag="h2T")
                for m2 in range(KD):
                    ps = pp.tile([P, 256], F32, tag="ps")
                    for hk in range(KD):
                        nc.tensor.matmul(out=ps[:], lhsT=n2w1[:, hk, ts(m2, P)],
                                         rhs=pooledT[:, hk, :],
                                         start=(hk == 0), stop=(hk == KD - 1))
                    nc.scalar.activation(out=h2T[:, m2, :], in_=ps[:], func=RELU,
                                         bias=n2b1[:, m2 : m2 + 1])
                return h2T

            def stage_f(b, h2T):
                # out_obj = relu(h2 @ n2w2 + n2b2)  (node-major)
                oo = op.tile([P, MN, DOUT], F32, tag="oo")
                oo_dram = out_obj_d[b].rearrange("(nm p) j -> p nm j", p=P)
                for nm in range(MN):
                    ps = pp.tile([P, 512], F32, tag="ps")
                    for k2 in range(KD):
                        nc.tensor.matmul(out=ps[:], lhsT=h2T[:, k2, ts(nm, P)],
                                         rhs=n2w2[:, k2, :],
                                         start=(k2 == 0),
                                         stop=(k2 == KD - 1 and not with_nb2))
                    if with_nb2:
                        nc.tensor.matmul(out=ps[:], lhsT=e0b[:], rhs=nb2pad[:],
                                         start=False, stop=True)
                    nc.scalar.activation(out=oo[:, nm, :], in_=ps[:], func=RELU)
                    nc.sync.dma_start(out=oo_dram[:, nm, :], in_=oo[:, nm, :])

            for b in range(BL):
                if b not in inputs_tl:
                    inputs_tl[b] = alloc_inputs(b)
                    load_inputs(b, inputs_tl[b])
                tl = inputs_tl.pop(b)
                if b in first_stage:
                    As, Ao = first_stage.pop(b)
                else:
                    As, Ao = stage_a(tl["objT"])
                hidT = stage_b(tl, As, Ao)
                ns, no = stage_c(b, hidT)
                pooledT = stage_d(tl, ns, no)
                h2T = stage_e(pooledT)
                stage_f(b, h2T)

    nc.compile()
    return nc


def _tile_kx(a):
    """[R, F] -> [128, R//128, F] with [p, k, f] = a[k*128+p, f], contiguous."""
    r, f = a.shape
    return np.ascontiguousarray(a.reshape(r // P, P, f).transpose(1, 0, 2))


def kernel(**inputs):
    obj = np.asarray(inputs["obj_vecs"], dtype=np.float32)    # [B, N, DO]
    pred = np.asarray(inputs["pred_vecs"], dtype=np.float32)  # [B, T, DP]
    edges = np.asarray(inputs["edges"])                       # [B, T, 3] int
    n1_w1 = np.asarray(inputs["n1_w1"], dtype=np.float32)     # [2DO+DP, H]
    n1_b1 = np.asarray(inputs["n1_b1"], dtype=np.float32)
    n1_w2 = np.asarray(inputs["n1_w2"], dtype=np.float32)     # [H, 2H+DOUT]
    n1_b2 = np.asarray(inputs["n1_b2"], dtype=np.float32)
    n2_w1 = np.asarray(inputs["n2_w1"], dtype=np.float32)
    n2_b1 = np.asarray(inputs["n2_b1"], dtype=np.float32)
    n2_w2 = np.asarray(inputs["n2_w2"], dtype=np.float32)
    n2_b2 = np.asarray(inputs["n2_b2"], dtype=np.float32)

    s_idx = edges[:, :, 0].astype(np.int64)  # [B, T]
    o_idx = edges[:, :, 2].astype(np.int64)

    # one-hots + per-node incidence counts (mean-pool scaling folded into scatter)
    ar_n = np.arange(N)
    gt_s = (s_idx[:, None, :] == ar_n[None, :, None])  # [B, N, T] bool
    gt_o = (o_idx[:, None, :] == ar_n[None, :, None])
    cnt = gt_s.sum(axis=2) + gt_o.sum(axis=2)          # [B, N]
    rcnt = 1.0 / np.maximum(cnt, 1).astype(np.float32)
    g_s = gt_s.transpose(0, 2, 1) * rcnt[:, None, :]   # [B, T, N] f32, scaled
    g_o = gt_o.transpose(0, 2, 1) * rcnt[:, None, :]

    # host tiling into [B, 128, chunks, free] bf16
    def tile_batch(a):  # [B, R, F] -> [B, 128, R//128, F]
        bsz, r, f = a.shape
        return np.ascontiguousarray(
            a.reshape(bsz, r // P, P, f).transpose(0, 2, 1, 3).astype(NPBF16)
        )

    objT_t = tile_batch(obj.transpose(0, 2, 1))   # [B,128,KD,N]
    predT_t = tile_batch(pred.transpose(0, 2, 1)) # [B,128,KD,T]
    gts_t = tile_batch(gt_s.astype(np.float32))   # [B,128,MN,T]
    gto_t = tile_batch(gt_o.astype(np.float32))
    gss_t = tile_batch(g_s)                       # [B,128,MT,N]
    gos_t = tile_batch(g_o)

    w1s_t = _tile_kx(n1_w1[0:DO]).astype(NPBF16)
    w1p_t = _tile_kx(n1_w1[DO : DO + DP]).astype(NPBF16)
    w1o_t = _tile_kx(n1_w1[DO + DP :]).astype(NPBF16)
    w2_t = _tile_kx(n1_w2).astype(NPBF16)
    n2w1_t = _tile_kx(n2_w1).astype(NPBF16)
    n2w2_t = _tile_kx(n2_w2).astype(NPBF16)
    b1_t = np.ascontiguousarray(n1_b1.reshape(KD, P).T)    # [128, KD] f32
    n2b1_t = np.ascontiguousarray(n2_b1.reshape(KD, P).T)

    with_b2 = bool(np.any(n1_b2 != 0))
    with_nb2 = bool(np.any(n2_b2 != 0))

    key = (with_b2, with_nb2)
    if key not in _PROGRAM_CACHE:
        _PROGRAM_CACHE[key] = _build_program(with_b2, with_nb2)
    nc = _PROGRAM_CACHE[key]

    in_maps = []
    for c in range(N_CORES):
        sl = slice(c * BL, (c + 1) * BL)
        m = {
            "objT": objT_t[sl], "predT": predT_t[sl],
            "gts": gts_t[sl], "gto": gto_t[sl],
            "gss": gss_t[sl], "gos": gos_t[sl],
            "w1s": w1s_t, "w1p": w1p_t, "w1o": w1o_t, "w2": w2_t,
            "n2w1": n2w1_t, "n2w2": n2w2_t, "b1": b1_t, "n2b1": n2b1_t,
        }
        if with_b2:
            e0 = np.zeros((P, P), NPBF16); e0[0, :] = 1
            m["e0"] = e0
            b2pad = np.zeros((P, 3, H), np.float32)
            b2pad[0, 0, :] = n1_b2[0:H]
            b2pad[0, 1, :] = n1_b2[H : H + DOUT]
            b2pad[0, 2, :] = n1_b2[H + DOUT :]
            m["b2pad"] = b2pad.astype(NPBF16)
        if with_nb2:
            e0b = np.zeros((P, P), NPBF16); e0b[0, :] = 1
            m["e0b"] = e0b
            nb2 = np.zeros((P, DOUT), np.float32); nb2[0, :] = n2_b2
            m["nb2pad"] = nb2.astype(NPBF16)
        in_maps.append(m)

    res = run_bass_kernel_spmd(nc, in_maps, core_ids=list(range(N_CORES)), trace=TRACE)
    global LAST_EXEC_TIME_NS
    LAST_EXEC_TIME_NS = res.exec_time_ns

    new_obj = np.empty((B, N, DOUT), np.float32)
    new_p = np.empty((B, T, DOUT), np.float32)
    for c in range(N_CORES):
        sl = slice(c * BL, (c + 1) * BL)
        new_obj[sl] = res.results[c]["out_obj"]
        new_p[sl] = res.results[c]["out_p"]
    return new_obj, new_p


# revision 25
# speedup vs baseline: 1.0074x; 1.0035x over previous
"""BatchGraphTripleConv on 8 TRN2 NeuronCores.

Data-parallel over batch B=64 (8 graphs per core). Gather/scatter are
expressed as one-hot matmuls on the TensorEngine:
  - gather:  cur_s.T-contribution = A_s.T @ Gt_s  where Gt_s[n,t] = [s_idx[t]==n]
  - scatter: pooled.T = new_s.T @ Gs + new_o.T @ Go where Gs[t,n] = [s_idx[t]==n]/cnt[n]
The mean-pool division is folded into the scatter one-hot on the host.
All matmuls run in bf16 with f32 PSUM accumulation; layouts are chosen so
no on-chip transpose is ever needed.
"""

import numpy as np
import ml_dtypes

import concourse.bacc as bacc
import concourse.mybir as mybir
import concourse.tile as tile
from concourse.bass import ts
from concourse.bass_utils import run_bass_kernel_spmd

B, N, T = 64, 256, 512
DO, DP, H, DOUT = 512, 512, 512, 512
N_CORES = 8
BL = B // N_CORES  # batch elems per core
P = 128
BF16 = mybir.dt.bfloat16
F32 = mybir.dt.float32
NPBF16 = ml_dtypes.bfloat16

KD = DO // P   # 4 k-chunks over feature dims (512)
MN = N // P    # 2 m-chunks over nodes (256)
MT = T // P    # 4 m-chunks over triples (512)

# dev knobs (unused by the grading path)
TRACE = False
LAST_EXEC_TIME_NS = None
_PROGRAM_CACHE = {}


def _build_program(with_b2: bool, with_nb2: bool):
    nc = bacc.Bacc(
        "TRN2",
        target_bir_lowering=False,
        debug=False,
        num_devices=N_CORES,
    )

    def din(name, shape, dt=BF16):
        return nc.dram_tensor(name, shape, dt, kind="ExternalInput").ap()

    # per-core batch shard, host pre-tiled to [BL, 128, chunks, free]
    objT_d = din("objT", [BL, P, KD, N])          # obj[b].T tiled: [p,k,n] = obj[b, n, k*128+p]
    predT_d = din("predT", [BL, P, KD, T])        # pred[b].T tiled
    gts_d = din("gts", [BL, P, MN, T])            # gather one-hot (subject): [p,kn,t] = [s_idx==kn*128+p]
    gto_d = din("gto", [BL, P, MN, T])
    gss_d = din("gss", [BL, P, MT, N])            # scatter one-hot / count: [p,tk,n]
    gos_d = din("gos", [BL, P, MT, N])
    # weights (same on every core)
    w1s_d = din("w1s", [P, KD, H])
    w1p_d = din("w1p", [P, KD, H])
    w1o_d = din("w1o", [P, KD, H])
    w2_d = din("w2", [P, KD, 2 * H + DOUT])
    n2w1_d = din("n2w1", [P, KD, H])
    n2w2_d = din("n2w2", [P, KD, DOUT])
    b1_d = din("b1", [P, KD], F32)
    n2b1_d = din("n2b1", [P, KD], F32)
    if with_b2:
        e0_d = din("e0", [P, P])                  # row0 = ones
        b2pad_d = din("b2pad", [P, 3, H])         # row0 = n1_b2 thirds
    if with_nb2:
        e0b_d = din("e0b", [P, P])
        nb2pad_d = din("nb2pad", [P, DOUT])       # row0 = n2_b2

    out_obj_d = nc.dram_tensor("out_obj", [BL, N, DOUT], F32, kind="ExternalOutput").ap()
    out_p_d = nc.dram_tensor("out_p", [BL, T, DOUT], F32, kind="ExternalOutput").ap()

    RELU = mybir.ActivationFunctionType.Relu

    with tile.TileContext(nc) as tc:
        with (
            tc.tile_pool(name="wpool", bufs=1) as wp,
            tc.tile_pool(name="inpool", bufs=2) as ip,
            tc.tile_pool(name="midpool", bufs=2) as mp,
            tc.tile_pool(name="outpool", bufs=2) as op,
            tc.tile_pool(name="pspool", bufs=8, space="PSUM") as pp,
        ):
            # ---- resident weights ----
            # Criticality-ordered DMA: the first stage-A matmuls need only
            # w1s/w1o k-chunk 0 and objT[0] k-chunk 0, so those stream first
            # (per-k granularity); everything not needed until stage B+ is
            # emitted after stage A so it overlaps compute.
            w1s = wp.tile([P, KD, H], BF16)
            w1p = wp.tile([P, KD, H], BF16)
            w1o = wp.tile([P, KD, H], BF16)
            w2 = wp.tile([P, KD, 2 * H + DOUT], BF16)
            n2w1 = wp.tile([P, KD, H], BF16)
            n2w2 = wp.tile([P, KD, DOUT], BF16)
            b1 = wp.tile([P, KD], F32)
            n2b1 = wp.tile([P, KD], F32)

            def alloc_inputs(b):
                return {
                    "objT": ip.tile([P, KD, N], BF16, tag="objT", name=f"objT{b}"),
                    "predT": ip.tile([P, KD, T], BF16, tag="predT", name=f"predT{b}"),
                    "gts": ip.tile([P, MN, T], BF16, tag="gts", name=f"gts{b}"),
                    "gto": ip.tile([P, MN, T], BF16, tag="gto", name=f"gto{b}"),
                    "gss": ip.tile([P, MT, N], BF16, tag="gss", name=f"gss{b}"),
                    "gos": ip.tile([P, MT, N], BF16, tag="gos", name=f"gos{b}"),
                }

            def load_inputs(b, tl):
                nc.sync.dma_start(out=tl["objT"][:], in_=objT_d[b])
                nc.sync.dma_start(out=tl["gts"][:], in_=gts_d[b])
                nc.sync.dma_start(out=tl["gto"][:], in_=gto_d[b])
                nc.sync.dma_start(out=tl["predT"][:], in_=predT_d[b])
                nc.sync.dma_start(out=tl["gss"][:], in_=gss_d[b])
                nc.sync.dma_start(out=tl["gos"][:], in_=gos_d[b])

            def stage_a(objT):
                # s-pass before o-pass: the first matmuls need only w1s chunks
                # + objT chunks, shrinking the b=0 critical DMA set; w1o
                # streams in behind the s-pass.
                As = mp.tile([P, MN, H], BF16, tag="As")
                Ao = mp.tile([P, MN, H], BF16, tag="Ao")
                for m in range(MN):
                    ps_s = pp.tile([P, 512], F32, tag="ps")
                    for k in range(KD):
                        nc.tensor.matmul(out=ps_s[:], lhsT=objT[:, k, ts(m, P)],
                                         rhs=w1s[:, k, :],
                                         start=(k == 0), stop=(k == KD - 1))
                    nc.vector.tensor_copy(out=As[:, m, :], in_=ps_s[:])
                for m in range(MN):
                    ps_o = pp.tile([P, 512], F32, tag="ps")
                    for k in range(KD):
                        nc.tensor.matmul(out=ps_o[:], lhsT=objT[:, k, ts(m, P)],
                                         rhs=w1o[:, k, :],
                                         start=(k == 0), stop=(k == KD - 1))
                    nc.vector.tensor_copy(out=Ao[:, m, :], in_=ps_o[:])
                return As, Ao

            # critical path: per-k interleaved loads of stage-A operands for b=0
            inputs_tl = {0: alloc_inputs(0), 1: alloc_inputs(1)}
            in0, in1 = inputs_tl[0], inputs_tl[1]
            for k in range(KD):
                nc.sync.dma_start(out=w1s[:, k, :], in_=w1s_d[:, k, :])
                nc.sync.dma_start(out=in0["objT"][:, k, :], in_=objT_d[0, :, k, :])
            for k in range(KD):
                nc.sync.dma_start(out=w1o[:, k, :], in_=w1o_d[:, k, :])
            nc.sync.dma_start(out=in1["objT"][:], in_=objT_d[1])
            nc.sync.dma_start(out=in0["gts"][:], in_=gts_d[0])
            nc.sync.dma_start(out=in0["gto"][:], in_=gto_d[0])
            nc.sync.dma_start(out=in0["predT"][:], in_=predT_d[0])
            nc.sync.dma_start(out=w1p[:], in_=w1p_d[:])

            # stage A for b=0 and b=1 back-to-back: PE work that covers the
            # stream-in of stage B/C inputs.
            first_stage = {0: stage_a(in0["objT"]), 1: stage_a(in1["objT"])}

            # deferred bulk weights: needed from stage B/C/E/F onward
            nc.sync.dma_start(out=b1[:], in_=b1_d[:])
            nc.sync.dma_start(out=w2[:], in_=w2_d[:])
            nc.sync.dma_start(out=in0["gss"][:], in_=gss_d[0])
            nc.sync.dma_start(out=in0["gos"][:], in_=gos_d[0])
            nc.sync.dma_start(out=n2w1[:], in_=n2w1_d[:])
            nc.sync.dma_start(out=n2w2[:], in_=n2w2_d[:])
            nc.sync.dma_start(out=n2b1[:], in_=n2b1_d[:])
            nc.sync.dma_start(out=in1["gts"][:], in_=gts_d[1])
            nc.sync.dma_start(out=in1["gto"][:], in_=gto_d[1])
            nc.sync.dma_start(out=in1["predT"][:], in_=predT_d[1])
            nc.sync.dma_start(out=in1["gss"][:], in_=gss_d[1])
            nc.sync.dma_start(out=in1["gos"][:], in_=gos_d[1])
            if with_b2:
                e0 = wp.tile([P, P], BF16)
                b2pad = wp.tile([P, 3, H], BF16)
                nc.sync.dma_start(out=e0[:], in_=e0_d[:])
                nc.sync.dma_start(out=b2pad[:], in_=b2pad_d[:])
            if with_nb2:
                e0b = wp.tile([P, P], BF16)
                nb2pad = wp.tile([P, DOUT], BF16)
                nc.sync.dma_start(out=e0b[:], in_=e0b_d[:])
                nc.sync.dma_start(out=nb2pad[:], in_=nb2pad_d[:])

            # ---- per-batch stage emitters (pairwise-interleaved below) ----
            def stage_b(tl, As, Ao):
                # hid.T = relu(A_s.T@Gt_s + A_o.T@Gt_o + W1_p.T@pred.T + b1);
                # gather MMs (need only gts/gto) before pred MMs (predT/w1p)
                hidT = mp.tile([P, KD, T], BF16, tag="hidT")
                ps_hid = []
                for hm in range(KD):
                    ps = pp.tile([P, 512], F32, tag="ps", name=f"ps_hid{hm}")
                    ps_hid.append(ps)
                    for kn in range(MN):
                        nc.tensor.matmul(out=ps[:], lhsT=As[:, kn, ts(hm, P)],
                                         rhs=tl["gts"][:, kn, :], start=(kn == 0), stop=False)
                        nc.tensor.matmul(out=ps[:], lhsT=Ao[:, kn, ts(hm, P)],
                                         rhs=tl["gto"][:, kn, :], start=False, stop=False)
                for hm in range(KD):
                    ps = ps_hid[hm]
                    for kp in range(KD):
                        nc.tensor.matmul(out=ps[:], lhsT=w1p[:, kp, ts(hm, P)],
                                         rhs=tl["predT"][:, kp, :],
                                         start=False, stop=(kp == KD - 1))
                    nc.scalar.activation(out=hidT[:, hm, :], in_=ps[:], func=RELU,
                                         bias=b1[:, hm : hm + 1])
                return hidT

            def stage_c(b, hidT):
                # new_t thirds, T-major: new_x = relu(hid @ W2_x + b2_x)
                ns = mp.tile([P, MT, H], BF16, tag="ns")
                no = mp.tile([P, MT, H], BF16, tag="no")
                npf = op.tile([P, MT, DOUT], F32, tag="npf")
                for tm in range(MT):
                    for third, dst in ((0, ns), (1, None), (2, no)):
                        ps = pp.tile([P, 512], F32, tag="ps")
                        for hk in range(KD):
                            nc.tensor.matmul(out=ps[:], lhsT=hidT[:, hk, ts(tm, P)],
                                             rhs=w2[:, hk, ts(third, 512)],
                                             start=(hk == 0),
                                             stop=(hk == KD - 1 and not with_b2))
                        if with_b2:
                            nc.tensor.matmul(out=ps[:], lhsT=e0[:], rhs=b2pad[:, third, :],
                                             start=False, stop=True)
                        if dst is None:
                            nc.scalar.activation(out=npf[:, tm, :], in_=ps[:], func=RELU)
                        else:
                            nc.scalar.activation(out=dst[:, tm, :], in_=ps[:], func=RELU)
                nc.sync.dma_start(
                    out=out_p_d[b].rearrange("(tm p) j -> p tm j", p=P), in_=npf[:]
                )
                return ns, no

            def stage_d(tl, ns, no):
                # pooled.T = new_s.T @ Gs + new_o.T @ Go  (count-scaled)
                pooledT = mp.tile([P, KD, N], BF16, tag="pooledT")
                for hm in range(KD):
                    ps = pp.tile([P, 256], F32, tag="ps")
                    for tk in range(MT):
                        nc.tensor.matmul(out=ps[:], lhsT=ns[:, tk, ts(hm, P)],
                                         rhs=tl["gss"][:, tk, :], start=(tk == 0), stop=False)
                        nc.tensor.matmul(out=ps[:], lhsT=no[:, tk, ts(hm, P)],
                                         rhs=tl["gos"][:, tk, :],
                                         start=False, stop=(tk == MT - 1))
                    nc.vector.tensor_copy(out=pooledT[:, hm, :], in_=ps[:])
                return pooledT

            def stage_e(pooledT):
                # h2.T = relu(n2w1.T @ pooled.T + n2b1)
                h2T = mp.tile([P, KD, N], BF16, tag="h2T")
                for m2 in range(KD):
                    ps = pp.tile([P, 256], F32, tag="ps")
                    for hk in range(KD):
                        nc.tensor.matmul(out=ps[:], lhsT=n2w1[:, hk, ts(m2, P)],
                                         rhs=pooledT[:, hk, :],
                                         start=(hk == 0), stop=(hk == KD - 1))
                    nc.scalar.activation(out=h2T[:, m2, :], in_=ps[:], func=RELU,
                                         bias=n2b1[:, m2 : m2 + 1])
                return h2T

            def stage_f(b, h2T):
                # out_obj = relu(h2 @ n2w2 + n2b2)  (node-major)
                oo = op.tile([P, MN, DOUT], F32, tag="oo")
                oo_dram = out_obj_d[b].rearrange("(nm p) j -> p nm j", p=P)
                for nm in range(MN):
                    ps = pp.tile([P, 512], F32, tag="ps")
                    for k2 in range(KD):
                        nc.tensor.matmul(out=ps[:], lhsT=h2T[:, k2, ts(nm, P)],
                                         rhs=n2w2[:, k2, :],
                                         start=(k2 == 0),
                                         stop=(k2 == KD - 1 and not with_nb2))
                    if with_nb2:
                        nc.tensor.matmul(out=ps[:], lhsT=e0b[:], rhs=nb2pad[:],
                                         start=False, stop=True)
                    nc.scalar.activation(out=oo[:, nm, :], in_=ps[:], func=RELU)
                    nc.sync.dma_start(out=oo_dram[:, nm, :], in_=oo[:, nm, :])

            for b in range(BL):
                if b not in inputs_tl:
                    inputs_tl[b] = alloc_inputs(b)
                    load_inputs(b, inputs_tl[b])
                tl = inputs_tl.pop(b)
                if b in first_stage:
                    As, Ao = first_stage.pop(b)
                else:
                    As, Ao = stage_a(tl["objT"])
                hidT = stage_b(tl, As, Ao)
                ns, no = stage_c(b, hidT)
                pooledT = stage_d(tl, ns, no)
                h2T = stage_e(pooledT)
                stage_f(b, h2T)

    nc.compile()
    return nc


def _tile_kx(a):
    """[R, F] -> [128, R//128, F] with [p, k, f] = a[k*128+p, f], contiguous."""
    r, f = a.shape
    return np.ascontiguousarray(a.reshape(r // P, P, f).transpose(1, 0, 2))


def kernel(**inputs):
    obj = np.asarray(inputs["obj_vecs"], dtype=np.float32)    # [B, N, DO]
    pred = np.asarray(inputs["pred_vecs"], dtype=np.float32)  # [B, T, DP]
    edges = np.asarray(inputs["edges"])                       # [B, T, 3] int
    n1_w1 = np.asarray(inputs["n1_w1"], dtype=np.float32)     # [2DO+DP, H]
    n1_b1 = np.asarray(inputs["n1_b1"], dtype=np.float32)
    n1_w2 = np.asarray(inputs["n1_w2"], dtype=np.float32)     # [H, 2H+DOUT]
    n1_b2 = np.asarray(inputs["n1_b2"], dtype=np.float32)
    n2_w1 = np.asarray(inputs["n2_w1"], dtype=np.float32)
    n2_b1 = np.asarray(inputs["n2_b1"], dtype=np.float32)
    n2_w2 = np.asarray(inputs["n2_w2"], dtype=np.float32)
    n2_b2 = np.asarray(inputs["n2_b2"], dtype=np.float32)

    s_idx = edges[:, :, 0].astype(np.int64)  # [B, T]
    o_idx = edges[:, :, 2].astype(np.int64)

    # one-hots + per-node incidence counts (mean-pool scaling folded into scatter)
    ar_n = np.arange(N)
    gt_s = (s_idx[:, None, :] == ar_n[None, :, None])  # [B, N, T] bool
    gt_o = (o_idx[:, None, :] == ar_n[None, :, None])
    cnt = gt_s.sum(axis=2) + gt_o.sum(axis=2)          # [B, N]
    rcnt = 1.0 / np.maximum(cnt, 1).astype(np.float32)
    g_s = gt_s.transpose(0, 2, 1) * rcnt[:, None, :]   # [B, T, N] f32, scaled
    g_o = gt_o.transpose(0, 2, 1) * rcnt[:, None, :]

    # host tiling into [B, 128, chunks, free] bf16
    def tile_batch(a):  # [B, R, F] -> [B, 128, R//128, F]
        bsz, r, f = a.shape
        return np.ascontiguousarray(
            a.reshape(bsz, r // P, P, f).transpose(0, 2, 1, 3).astype(NPBF16)
        )

    objT_t = tile_batch(obj.transpose(0, 2, 1))   # [B,128,KD,N]
    predT_t = tile_batch(pred.transpose(0, 2, 1)) # [B,128,KD,T]
    gts_t = tile_batch(gt_s.astype(np.float32))   # [B,128,MN,T]
    gto_t = tile_batch(gt_o.astype(np.float32))
    gss_t = tile_batch(g_s)                       # [B,128,MT,N]
    gos_t = tile_batch(g_o)

    w1s_t = _tile_kx(n1_w1[0:DO]).astype(NPBF16)
    w1p_t = _tile_kx(n1_w1[DO : DO + DP]).astype(NPBF16)
    w1o_t = _tile_kx(n1_w1[DO + DP :]).astype(NPBF16)
    w2_t = _tile_kx(n1_w2).astype(NPBF16)
    n2w1_t = _tile_kx(n2_w1).astype(NPBF16)
    n2w2_t = _tile_kx(n2_w2).astype(NPBF16)
    b1_t = np.ascontiguousarray(n1_b1.reshape(KD, P).T)    # [128, KD] f32
    n2b1_t = np.ascontiguousarray(n2_b1.reshape(KD, P).T)

    with_b2 = bool(np.any(n1_b2 != 0))
    with_nb2 = bool(np.any(n2_b2 != 0))

    key = (with_b2, with_nb2)
    if key not in _PROGRAM_CACHE:
        _PROGRAM_CACHE[key] = _build_program(with_b2, with_nb2)
    nc = _PROGRAM_CACHE[key]

    in_maps = []
    for c in range(N_CORES):
        sl = slice(c * BL, (c + 1) * BL)
        m = {
            "objT": objT_t[sl], "predT": predT_t[sl],
            "gts": gts_t[sl], "gto": gto_t[sl],
            "gss": gss_t[sl], "gos": gos_t[sl],
            "w1s": w1s_t, "w1p": w1p_t, "w1o": w1o_t, "w2": w2_t,
            "n2w1": n2w1_t, "n2w2": n2w2_t, "b1": b1_t, "n2b1": n2b1_t,
        }
        if with_b2:
            e0 = np.zeros((P, P), NPBF16); e0[0, :] = 1
            m["e0"] = e0
            b2pad = np.zeros((P, 3, H), np.float32)
            b2pad[0, 0, :] = n1_b2[0:H]
            b2pad[0, 1, :] = n1_b2[H : H + DOUT]
            b2pad[0, 2, :] = n1_b2[H + DOUT :]
            m["b2pad"] = b2pad.astype(NPBF16)
        if with_nb2:
            e0b = np.zeros((P, P), NPBF16); e0b[0, :] = 1
            m["e0b"] = e0b
            nb2 = np.zeros((P, DOUT), np.float32); nb2[0, :] = n2_b2
            m["nb2pad"] = nb2.astype(NPBF16)
        in_maps.append(m)

    res = run_bass_kernel_spmd(nc, in_maps, core_ids=list(range(N_CORES)), trace=TRACE)
    global LAST_EXEC_TIME_NS
    LAST_EXEC_TIME_NS = res.exec_time_ns

    new_obj = np.empty((B, N, DOUT), np.float32)
    new_p = np.empty((B, T, DOUT), np.float32)
    for c in range(N_CORES):
        sl = slice(c * BL, (c + 1) * BL)
        new_obj[sl] = res.results[c]["out_obj"]
        new_p[sl] = res.results[c]["out_p"]
    return new_obj, new_p
